# revision 6
# baseline (speedup 1.0000x reference)
"""Trainium2 Bass kernel for a CPPN-style dense MLP forward pass.

Network (per pixel): 11 -> [32 x 23 tanh layers] -> 3 sigmoid.
  h = tanh(x @ W1.T); 22x: h = tanh(h @ Whid[l].T); out = sigmoid(h @ Wout.T)

Full inputs:  x [4194304, 11] f32, W1 [32, 11], Whid [22, 32, 32], Wout [3, 32]
Full output:  [4194304, 3] f32

Strategy: pure data parallel over 8 NeuronCores (pixels split 8 ways,
weights replicated).  Per core the kernel is ScalarE(tanh)-throughput
bound (24 layers x 32 feats x 512K pixels / 128 lanes at 1.2 GHz =
2.62 ms of pure ACT streaming).  Two levers beyond the ACT floor:

1. ~11% of the tanh tiles are offloaded to the otherwise-idle VectorE
   via a 6-instruction fp32 chain: clamp (tensor_scalar), a (9,4)
   rational x*P(x^2)/D(x^2) evaluated with two custom DVE uop programs
   (8-deep ALU pipe, 1 elem/cycle), the stock bitwise-NOT+2xNR fast
   reciprocal (~51 ULP), and a tensor_tensor multiply.  Max abs error
   1.1e-5 per application; end-to-end L2 ~3e-3 at 1/8 offload (gate
   2e-2).  Offloaded tiles vacate PSUM immediately (the clamp is the
   only PSUM reader), and FOUR supertile streams rotate over the two
   PSUM halves so ACT+PE stay busy during the ~13us DVE windows.

2. Pair-boundary DMA/transpose bursts (which perturbed the PE enough
   to stall ACT ~9% of the time in the 2-stream baseline) are spread:
   loads for the next group, input transposes, output transposes and
   stores are emitted one per (layer,stream) slot via a deferred queue.

Layout per core: pixels in supertiles of 16 tiles x 512 = 8192 pixels.
Activations are feature-major: tile (a,b) holds [32 feats, 512 pixels]
at partitions [32u:32u+32], free offset 512*v, (u,v)=(a,b) on even
layers, (b,a) on odd.  Each layer = 16 concurrent 32x32 matmuls via
tile_position packing writing one [128,2048] PSUM half, then one ACT
tanh (or the DVE chain) to SBUF.  Matmuls are full fp32 (2-pass
LOW/HIGH): the 24-layer tanh chain amplifies per-layer error ~1000x,
so reduced matmul precision (f32r: 0.14 L2) is unusable.

I/O: x loads pixel-major with 44B contiguous chunks, block-transposed
feature-major on VectorE (32x32 STREAM_TRANSPOSE); sigmoid output is
block-transposed back so stores scatter 12B/pixel chunks.  All DMA is
issued from SyncE, spread across layer slots.
"""

import os
import sys

if "/opt/trn_rl_repo" not in sys.path:
    sys.path.insert(0, "/opt/trn_rl_repo")

import numpy as np

N_CORES = 8
N_PIX = 4194304
P_CORE = N_PIX // N_CORES      # 524288 pixels per core
D_IN = 11
D_H = 32
N_LAYERS = 24                  # 1 input + 22 hidden + 1 output
F = 512                        # pixels per tile (one PSUM bank of fp32)
ST_PIX = 16 * F                # 8192 pixels per supertile
N_ST = P_CORE // ST_PIX        # 64 supertiles per core
G = 4                          # supertile streams rotating per group
N_GROUPS = N_ST // G           # 16 groups per core
G_PIX = G * ST_PIX             # 32768 pixels per group

# (9,4) odd rational tanh(x) ~ xc*P(s)/D(s), s=xc^2, xc=clamp(x,+-CLAMP).
# Max abs err 8.1e-6 (fit) / 1.11e-5 (fp32 chain incl ~51ULP reciprocal).
TANH_CLAMP = 7.90531
TA1 = 9.99963490e-01
TA3 = 1.10930869e-01
TA5 = 1.11048222e-03
TA7 = -3.85775571e-06
TA9 = 1.18000171e-08
TB2 = 0.44415242
TB4 = 0.01593033

_BUILD_CACHE = {}
_DVE_OPS = {}


def _register_dve_ops():
    """Register the two custom DVE uop programs used by the tanh chain.

    POLY2_ANT: out = (s*C0 + C1)*s + C2, s = in0^2   (numerator partial
      with (C0,C1,C2)=(a9,a7,a5); denominator with (b4,b2,1.0))
    TANH_NUMB_ANT: out = ((in1*s + C0)*s + C1)*in0, s = in0^2
      (finishes the odd numerator: N = ((A*s+a3)*s+a1)*xc)
    """
    if _DVE_OPS:
        return _DVE_OPS
    import concourse.dve_ops as dve_ops
    from concourse.dve_spec import Spec, Src0, Src1, C0, C1, C2, sq, lower
    from concourse.dve_spec import _has_src1
    from concourse.dve_uop import DveOpSpec

    def make(name, spec):
        shas = {}
        for ver in ("v3", "v4"):
            d = DveOpSpec(
                name=name, opcode=1, uops=lower(spec, ver=ver),
                rd1_en=_has_src1(spec),
            )
            shas[ver] = d.sha(ver)
        op = dve_ops.DveOp(name=name, spec=spec, subdim=False, uops_sha=shas)
        if name not in dve_ops._SUB_OPCODE_FOR_NAME:
            dve_ops.OPS.append(op)
            dve_ops._SUB_OPCODE_FOR_NAME[name] = (
                dve_ops._CUSTOM_DVE_ROW_BASE + len(dve_ops.OPS) - 1
            )
            dve_ops.CUSTOM_DVE_SPECS[name] = spec
        assert dve_ops._SUB_OPCODE_FOR_NAME[name] < 0x20
        return op

    s = sq(Src0)
    poly2 = Spec(
        body=(s * C0 + C1) * s + C2,
        reference=lambda in0, in1, c0, c1, c2: (
            ((in0 * in0) * c0 + c1) * (in0 * in0) + c2
        ),
    )
    s2 = sq(Src0)
    numb = Spec(
        body=((Src1 * s2 + C0) * s2 + C1) * Src0,
        reference=lambda in0, in1, c0, c1, c2: (
            ((in1 * (in0 * in0) + c0) * (in0 * in0) + c1) * in0
        ),
    )
    _DVE_OPS["poly2"] = make("POLY2_ANT", poly2)
    _DVE_OPS["numb"] = make("TANH_NUMB_ANT", numb)
    return _DVE_OPS


def _build(n_groups, offp):
    """Build + bass-compile the per-core program. Returns the Bacc object."""
    import concourse.bass as bass  # noqa: F401
    import concourse.tile as tile
    from concourse import bacc, mybir
    from concourse.alu_op_type import AluOpType
    from contextlib import ExitStack

    ops = _register_dve_ops()
    f32 = mybir.dt.float32
    Tanh = mybir.ActivationFunctionType.Tanh
    Sigmoid = mybir.ActivationFunctionType.Sigmoid

    nc = bacc.Bacc(
        "TRN2", target_bir_lowering=False, debug=False, num_devices=N_CORES
    )
    x_ap = nc.dram_tensor("x", [P_CORE, D_IN], f32, kind="ExternalInput").ap()
    w_ap = nc.dram_tensor("w", [128, N_LAYERS * 32], f32, kind="ExternalInput").ap()
    o_ap = nc.dram_tensor("o", [P_CORE, 3], f32, kind="ExternalOutput").ap()

    with tile.TileContext(nc) as tc, ExitStack() as ctx:
        wp = ctx.enter_context(tc.tile_pool(name="wp", bufs=1))
        xrp = ctx.enter_context(tc.tile_pool(name="xrp", bufs=4))
        xp = ctx.enter_context(tc.tile_pool(name="xp", bufs=4))
        hp = ctx.enter_context(tc.tile_pool(name="hp", bufs=5))
        op_ = ctx.enter_context(tc.tile_pool(name="op", bufs=4))
        srp = ctx.enter_context(tc.tile_pool(name="srp", bufs=2))
        cp = ctx.enter_context(tc.tile_pool(name="cp", bufs=5))
        pp = ctx.enter_context(tc.tile_pool(name="pp", bufs=2, space="PSUM"))

        Wf = wp.tile([128, N_LAYERS * 32], f32)
        nc.sync.dma_start(Wf[:], w_ap[:])

        # ---- deferred work queue: one item per (layer, stream) slot ----
        deferred = []

        def pump():
            if deferred:
                deferred.pop(0)()

        def emit_load_dma(s, u):
            # Pixel-major load, 44B contiguous chunks per pixel row, laid
            # out so a 32x32 block-transpose yields feature-major tiles:
            # XR[32u+p, 32c+f] = x[s*8192 + u*2048 + 32c + p, f].
            XR = xr_tiles[s % 8]
            p0 = s * ST_PIX + u * 2048
            nc.sync.dma_start(
                XR[32 * u : 32 * u + 32, :].rearrange(
                    "p (c f) -> p c f", c=64, f=32
                )[:, :, 0:D_IN],
                x_ap[p0 : p0 + 2048, :].rearrange("(c p) f -> p c f", c=64, p=32),
            )

        def emit_transpose(s):
            # DVE 32x32 block transpose -> X[32u+f, 32c+p].
            X = xp.tile([128, 2048], f32, name="X", tag="X")
            nc.vector.transpose(X[:], xr_tiles[s % 8][:])
            x_tiles[s % 8] = X

        xr_tiles = {}
        x_tiles = {}

        def alloc_xr(s):
            xr_tiles[s % 8] = xrp.tile([128, 2048], f32, name="XR", tag="XR")

        def emit_store(s, S):
            # Block-transpose back to pixel-major; 12B/pixel store chunks.
            # Both the transpose and the 4 store DMAs are deferred so they
            # spread across the next group's layer slots (and the srp pool
            # never head-of-line-blocks the DVE queue).
            box = {}

            def tp(S=S, box=box):
                SR = srp.tile([128, 2048], f32, name="SR", tag="SR")
                nc.vector.transpose(SR[:], S[:])
                box["SR"] = SR

            deferred.append(tp)
            for a in range(4):
                p0 = s * ST_PIX + a * 2048

                def st(a=a, p0=p0, box=box):
                    SR = box["SR"]
                    nc.sync.dma_start(
                        o_ap[p0 : p0 + 2048, :].rearrange(
                            "(c p) f -> p c f", c=64, p=32
                        ),
                        SR[32 * a : 32 * a + 32, :].rearrange(
                            "p (c f) -> p c f", c=64, f=32
                        )[:, :, 0:3],
                    )

                deferred.append(st)

        counter = [0]

        def layer(H, k, offload):
            """One layer for one supertile: 16 packed matmuls + tanh."""
            Kd = D_IN if k == 0 else 32
            last = k == N_LAYERS - 1
            P_ = pp.tile([128, 2048], f32)
            # fp32 16-way tile-position packing; iterate so consecutive
            # matmuls land on different PE row groups (LDWEIGHTS only
            # pulls ahead of in-flight MMs when row_grp differs).
            ab = [(a, b) for b in range(4) for a in range(4)]
            if k % 2 == 1:
                ab = [(a, b) for a in range(4) for b in range(4)]
            for a, b in ab:
                u, v = (a, b) if k % 2 == 0 else (b, a)
                nc.tensor.matmul(
                    P_[32 * v : 32 * v + 32, 512 * u : 512 * u + 512],
                    lhsT=Wf[32 * u : 32 * u + Kd, 32 * k : 32 * k + 32],
                    rhs=H[32 * u : 32 * u + Kd, 512 * v : 512 * v + 512],
                    start=True,
                    stop=True,
                    tile_position=(32 * u, 32 * v),
                )
            if last:
                Hn = op_.tile([128, 2048], f32)
                nc.scalar.activation(Hn[:], P_[:], Sigmoid)
            elif offload:
                mn, mx = AluOpType.min, AluOpType.max
                xc = cp.tile([128, 2048], f32, name="xc", tag="chain")
                nc.vector.tensor_scalar(
                    xc[:], P_[:], float(TANH_CLAMP), float(-TANH_CLAMP), mn, mx
                )
                A = cp.tile([128, 2048], f32, name="A", tag="chain")
                nc.vector._custom_dve(
                    ops["poly2"], out=A[:], in0=xc[:],
                    s0=float(TA9), s1=float(TA7), imm2=float(TA5),
                )
                Nt = cp.tile([128, 2048], f32, name="Nt", tag="chain")
                nc.vector._custom_dve(
                    ops["numb"], out=Nt[:], in0=xc[:], in1=A[:],
                    s0=float(TA3), s1=float(TA1),
                )
                Dt = cp.tile([128, 2048], f32, name="Dt", tag="chain")
                nc.vector._custom_dve(
                    ops["poly2"], out=Dt[:], in0=xc[:],
                    s0=float(TB4), s1=float(TB2), imm2=1.0,
                )
                Rt = cp.tile([128, 2048], f32, name="Rt", tag="chain")
                nc.vector.reciprocal_approx_fast(Rt[:], Dt[:])
                Hn = hp.tile([128, 2048], f32)
                nc.vector.tensor_tensor(Hn[:], Nt[:], Rt[:], AluOpType.mult)
            else:
                Hn = hp.tile([128, 2048], f32)
                nc.scalar.activation(Hn[:], P_[:], Tanh)
            return Hn

        # ---- prologue: group 0 loads + transposes ----
        for j in range(G):
            alloc_xr(j)
            for u in range(4):
                emit_load_dma(j, u)
        for j in range(G):
            emit_transpose(j)

        for g in range(n_groups):
            s0 = g * G
            H = [x_tiles[(s0 + j) % 8] for j in range(G)]
            # queue next group's loads (16 items) then transposes (4)
            if g + 1 < n_groups:
                for j in range(G):
                    sn = s0 + G + j
                    deferred.append(lambda sn=sn: alloc_xr(sn))
                    for u in range(4):
                        deferred.append(
                            lambda sn=sn, u=u: emit_load_dma(sn, u)
                        )
                for j in range(G):
                    sn = s0 + G + j
                    deferred.append(lambda sn=sn: emit_transpose(sn))
            for k in range(N_LAYERS):
                for j in range(G):
                    off = False
                    if k < N_LAYERS - 1:
                        off = counter[0] % offp == 0
                        counter[0] += 1
                    H[j] = layer(H[j], k, off)
                    pump()
            for j in range(G):
                emit_store(s0 + j, H[j])

        while deferred:
            pump()

    nc.compile()
    return nc


def _get_program(n_groups, offp):
    key = (n_groups, offp)
    if key not in _BUILD_CACHE:
        _BUILD_CACHE[key] = _build(n_groups, offp)
    return _BUILD_CACHE[key]


def _pack_weights(W1, Whid, Wout):
    """[128, 24*32]: per partition-group u, column block l*32 holds W_l.T."""
    WT = np.zeros((N_LAYERS, 32, 32), np.float32)
    WT[0, :D_IN, :] = np.asarray(W1, np.float32).T
    WT[1:23] = np.transpose(np.asarray(Whid, np.float32), (0, 2, 1))
    WT[23, :, :3] = np.asarray(Wout, np.float32).T
    Wh = np.zeros((128, N_LAYERS * 32), np.float32)
    blocks = WT.transpose(1, 0, 2).reshape(32, N_LAYERS * 32)
    for u in range(4):
        Wh[32 * u : 32 * u + 32, :] = blocks
    return Wh


def _run(x, W1, Whid, Wout, trace=False, n_groups=None, **spmd_kwargs):
    from concourse.bass_utils import run_bass_kernel_spmd

    if n_groups is None:
        n_groups = int(os.environ.get("BASSK_GROUPS", N_GROUPS))
    offp = int(os.environ.get("BASSK_OFFP", 9))
    nc = _get_program(n_groups, offp)

    x = np.ascontiguousarray(np.asarray(x, np.float32))
    assert x.shape == (N_PIX, D_IN), x.shape
    Wh = _pack_weights(W1, Whid, Wout)

    in_maps = [
        {"x": x[i * P_CORE : (i + 1) * P_CORE], "w": Wh}
        for i in range(N_CORES)
    ]
    res = run_bass_kernel_spmd(
        nc, in_maps, list(range(N_CORES)), trace=trace, **spmd_kwargs
    )
    out = np.concatenate([res.results[i]["o"] for i in range(N_CORES)], axis=0)
    return out, res


def kernel(x, W1, Whid, Wout):
    out, _ = _run(x, W1, Whid, Wout)
    return out


# revision 9
# speedup vs baseline: 1.0213x; 1.0213x over previous
"""Trainium2 Bass kernel for a CPPN-style dense MLP forward pass.

Network (per pixel): 11 -> [32 x 23 tanh layers] -> 3 sigmoid.
  h = tanh(x @ W1.T); 22x: h = tanh(h @ Whid[l].T); out = sigmoid(h @ Wout.T)

Full inputs:  x [4194304, 11] f32, W1 [32, 11], Whid [22, 32, 32], Wout [3, 32]
Full output:  [4194304, 3] f32

Strategy: pure data parallel over 8 NeuronCores (pixels split 8 ways,
weights replicated).  Per core the kernel is ScalarE(tanh)-throughput
bound (24 layers x 32 feats x 512K pixels / 128 lanes at 1.2 GHz =
2.62 ms of pure ACT streaming).  Two levers beyond the ACT floor:

1. ~11% of the tanh tiles are offloaded to the otherwise-idle VectorE
   via a 6-instruction fp32 chain: clamp (tensor_scalar), a (9,4)
   rational x*P(x^2)/D(x^2) evaluated with two custom DVE uop programs
   (8-deep ALU pipe, 1 elem/cycle), the stock bitwise-NOT+2xNR fast
   reciprocal (~51 ULP), and a tensor_tensor multiply.  Max abs error
   1.1e-5 per application; end-to-end L2 ~3e-3 at 1/8 offload (gate
   2e-2).  Offloaded tiles vacate PSUM immediately (the clamp is the
   only PSUM reader), and FOUR supertile streams rotate over the two
   PSUM halves so ACT+PE stay busy during the ~13us DVE windows.

2. Pair-boundary DMA/transpose bursts (which perturbed the PE enough
   to stall ACT ~9% of the time in the 2-stream baseline) are spread:
   loads for the next group, input transposes, output transposes and
   stores are emitted one per (layer,stream) slot via a deferred queue.

Layout per core: pixels in supertiles of 16 tiles x 512 = 8192 pixels.
Activations are feature-major: tile (a,b) holds [32 feats, 512 pixels]
at partitions [32u:32u+32], free offset 512*v, (u,v)=(a,b) on even
layers, (b,a) on odd.  Each layer = 16 concurrent 32x32 matmuls via
tile_position packing writing one [128,2048] PSUM half, then one ACT
tanh (or the DVE chain) to SBUF.  Matmuls are full fp32 (2-pass
LOW/HIGH): the 24-layer tanh chain amplifies per-layer error ~1000x,
so reduced matmul precision (f32r: 0.14 L2) is unusable.

I/O: x loads pixel-major with 44B contiguous chunks, block-transposed
feature-major on VectorE (32x32 STREAM_TRANSPOSE); sigmoid output is
block-transposed back so stores scatter 12B/pixel chunks.  All DMA is
issued from SyncE, spread across layer slots.
"""

import os
import sys

if "/opt/trn_rl_repo" not in sys.path:
    sys.path.insert(0, "/opt/trn_rl_repo")

import numpy as np

N_CORES = 8
N_PIX = 4194304
P_CORE = N_PIX // N_CORES      # 524288 pixels per core
D_IN = 11
D_H = 32
N_LAYERS = 24                  # 1 input + 22 hidden + 1 output
F = 512                        # pixels per tile (one PSUM bank of fp32)
ST_PIX = 16 * F                # 8192 pixels per supertile
N_ST = P_CORE // ST_PIX        # 64 supertiles per core
G = 4                          # supertile streams rotating per group
N_GROUPS = N_ST // G           # 16 groups per core
G_PIX = G * ST_PIX             # 32768 pixels per group

# (9,4) odd rational tanh(x) ~ xc*P(s)/D(s), s=xc^2, xc=clamp(x,+-CLAMP).
# Max abs err 8.1e-6 (fit) / 1.11e-5 (fp32 chain incl ~51ULP reciprocal).
TANH_CLAMP = 7.90531
TA1 = 9.99963490e-01
TA3 = 1.10930869e-01
TA5 = 1.11048222e-03
TA7 = -3.85775571e-06
TA9 = 1.18000171e-08
TB2 = 0.44415242
TB4 = 0.01593033

_BUILD_CACHE = {}
_DVE_OPS = {}


def _register_dve_ops():
    """Register the two custom DVE uop programs used by the tanh chain.

    POLY2_ANT: out = (s*C0 + C1)*s + C2, s = in0^2   (numerator partial
      with (C0,C1,C2)=(a9,a7,a5); denominator with (b4,b2,1.0))
    TANH_NUMB_ANT: out = ((in1*s + C0)*s + C1)*in0, s = in0^2
      (finishes the odd numerator: N = ((A*s+a3)*s+a1)*xc)
    """
    if _DVE_OPS:
        return _DVE_OPS
    import concourse.dve_ops as dve_ops
    from concourse.dve_spec import Spec, Src0, Src1, C0, C1, C2, sq, lower
    from concourse.dve_spec import _has_src1
    from concourse.dve_uop import DveOpSpec

    def make(name, spec):
        shas = {}
        for ver in ("v3", "v4"):
            d = DveOpSpec(
                name=name, opcode=1, uops=lower(spec, ver=ver),
                rd1_en=_has_src1(spec),
            )
            shas[ver] = d.sha(ver)
        op = dve_ops.DveOp(name=name, spec=spec, subdim=False, uops_sha=shas)
        if name not in dve_ops._SUB_OPCODE_FOR_NAME:
            dve_ops.OPS.append(op)
            dve_ops._SUB_OPCODE_FOR_NAME[name] = (
                dve_ops._CUSTOM_DVE_ROW_BASE + len(dve_ops.OPS) - 1
            )
            dve_ops.CUSTOM_DVE_SPECS[name] = spec
        assert dve_ops._SUB_OPCODE_FOR_NAME[name] < 0x20
        return op

    s = sq(Src0)
    poly2 = Spec(
        body=(s * C0 + C1) * s + C2,
        reference=lambda in0, in1, c0, c1, c2: (
            ((in0 * in0) * c0 + c1) * (in0 * in0) + c2
        ),
    )
    s2 = sq(Src0)
    numb = Spec(
        body=((Src1 * s2 + C0) * s2 + C1) * Src0,
        reference=lambda in0, in1, c0, c1, c2: (
            ((in1 * (in0 * in0) + c0) * (in0 * in0) + c1) * in0
        ),
    )
    _DVE_OPS["poly2"] = make("POLY2_ANT", poly2)
    _DVE_OPS["numb"] = make("TANH_NUMB_ANT", numb)
    return _DVE_OPS


def _build(n_groups, offp):
    """Build + bass-compile the per-core program. Returns the Bacc object."""
    import concourse.bass as bass  # noqa: F401
    import concourse.tile as tile
    from concourse import bacc, mybir
    from concourse.alu_op_type import AluOpType
    from contextlib import ExitStack

    ops = _register_dve_ops()
    f32 = mybir.dt.float32
    Tanh = mybir.ActivationFunctionType.Tanh
    Sigmoid = mybir.ActivationFunctionType.Sigmoid

    nc = bacc.Bacc(
        "TRN2", target_bir_lowering=False, debug=False, num_devices=N_CORES
    )
    x_ap = nc.dram_tensor("x", [P_CORE, D_IN], f32, kind="ExternalInput").ap()
    w_ap = nc.dram_tensor("w", [128, N_LAYERS * 32], f32, kind="ExternalInput").ap()
    o_ap = nc.dram_tensor("o", [P_CORE, 3], f32, kind="ExternalOutput").ap()

    with tile.TileContext(nc) as tc, ExitStack() as ctx:
        wp = ctx.enter_context(tc.tile_pool(name="wp", bufs=1))
        xrp = ctx.enter_context(tc.tile_pool(name="xrp", bufs=4))
        xp = ctx.enter_context(tc.tile_pool(name="xp", bufs=4))
        hp = ctx.enter_context(tc.tile_pool(name="hp", bufs=5))
        op_ = ctx.enter_context(tc.tile_pool(name="op", bufs=4))
        srp = ctx.enter_context(tc.tile_pool(name="srp", bufs=2))
        cp = ctx.enter_context(tc.tile_pool(name="cp", bufs=5))
        pp = ctx.enter_context(tc.tile_pool(name="pp", bufs=2, space="PSUM"))

        Wf = wp.tile([128, N_LAYERS * 32], f32)
        nc.sync.dma_start(Wf[:], w_ap[:])

        # ---- deferred work queue: one item per (layer, stream) slot ----
        deferred = []

        def pump():
            if deferred:
                deferred.pop(0)()

        def emit_load_dma(s, u, eng=None):
            # Pixel-major load, 44B contiguous chunks per pixel row, laid
            # out so a 32x32 block-transpose yields feature-major tiles:
            # XR[32u+p, 32c+f] = x[s*8192 + u*2048 + 32c + p, f].
            XR = xr_tiles[s % 8]
            p0 = s * ST_PIX + u * 2048
            (eng or nc.sync).dma_start(
                XR[32 * u : 32 * u + 32, :].rearrange(
                    "p (c f) -> p c f", c=64, f=32
                )[:, :, 0:D_IN],
                x_ap[p0 : p0 + 2048, :].rearrange("(c p) f -> p c f", c=64, p=32),
            )

        def emit_transpose(s):
            # DVE 32x32 block transpose -> X[32u+f, 32c+p].
            X = xp.tile([128, 2048], f32, name="X", tag="X")
            nc.vector.transpose(X[:], xr_tiles[s % 8][:])
            x_tiles[s % 8] = X

        xr_tiles = {}
        x_tiles = {}

        def alloc_xr(s):
            xr_tiles[s % 8] = xrp.tile([128, 2048], f32, name="XR", tag="XR")

        def emit_store(s, S, out):
            # Block-transpose back to pixel-major; 12B/pixel store chunks.
            # Both the transpose and the 4 store DMAs are deferred so they
            # spread across the next group's layer slots.  Stores issue on
            # the (otherwise idle) GpSimd queue so they never delay the
            # next group's input loads on SyncE.
            box = {}

            def tp(S=S, box=box):
                SR = srp.tile([128, 2048], f32, name="SR", tag="SR")
                nc.vector.transpose(SR[:], S[:])
                box["SR"] = SR

            out.append(tp)
            for a in range(4):
                p0 = s * ST_PIX + a * 2048

                def st(a=a, p0=p0, box=box):
                    SR = box["SR"]
                    nc.gpsimd.dma_start(
                        o_ap[p0 : p0 + 2048, :].rearrange(
                            "(c p) f -> p c f", c=64, p=32
                        ),
                        SR[32 * a : 32 * a + 32, :].rearrange(
                            "p (c f) -> p c f", c=64, f=32
                        )[:, :, 0:3],
                    )

                out.append(st)

        counter = [0]

        def layer(H, k, offload):
            """One layer for one supertile: 16 packed matmuls + tanh."""
            Kd = D_IN if k == 0 else 32
            last = k == N_LAYERS - 1
            P_ = pp.tile([128, 2048], f32)
            # fp32 16-way tile-position packing; iterate so consecutive
            # matmuls land on different PE row groups (LDWEIGHTS only
            # pulls ahead of in-flight MMs when row_grp differs).
            ab = [(a, b) for b in range(4) for a in range(4)]
            if k % 2 == 1:
                ab = [(a, b) for a in range(4) for b in range(4)]
            for a, b in ab:
                u, v = (a, b) if k % 2 == 0 else (b, a)
                nc.tensor.matmul(
                    P_[32 * v : 32 * v + 32, 512 * u : 512 * u + 512],
                    lhsT=Wf[32 * u : 32 * u + Kd, 32 * k : 32 * k + 32],
                    rhs=H[32 * u : 32 * u + Kd, 512 * v : 512 * v + 512],
                    start=True,
                    stop=True,
                    tile_position=(32 * u, 32 * v),
                )
            if last:
                Hn = op_.tile([128, 2048], f32)
                nc.scalar.activation(Hn[:], P_[:], Sigmoid)
            elif offload:
                mn, mx = AluOpType.min, AluOpType.max
                xc = cp.tile([128, 2048], f32, name="xc", tag="chain")
                nc.vector.tensor_scalar(
                    xc[:], P_[:], float(TANH_CLAMP), float(-TANH_CLAMP), mn, mx
                )
                A = cp.tile([128, 2048], f32, name="A", tag="chain")
                nc.vector._custom_dve(
                    ops["poly2"], out=A[:], in0=xc[:],
                    s0=float(TA9), s1=float(TA7), imm2=float(TA5),
                )
                Nt = cp.tile([128, 2048], f32, name="Nt", tag="chain")
                nc.vector._custom_dve(
                    ops["numb"], out=Nt[:], in0=xc[:], in1=A[:],
                    s0=float(TA3), s1=float(TA1),
                )
                Dt = cp.tile([128, 2048], f32, name="Dt", tag="chain")
                nc.vector._custom_dve(
                    ops["poly2"], out=Dt[:], in0=xc[:],
                    s0=float(TB4), s1=float(TB2), imm2=1.0,
                )
                Rt = cp.tile([128, 2048], f32, name="Rt", tag="chain")
                nc.vector.reciprocal_approx_fast(Rt[:], Dt[:])
                Hn = hp.tile([128, 2048], f32)
                nc.vector.tensor_tensor(Hn[:], Nt[:], Rt[:], AluOpType.mult)
            else:
                Hn = hp.tile([128, 2048], f32)
                nc.scalar.activation(Hn[:], P_[:], Tanh)
            return Hn

        # ---- prologue: group 0 loads (split across both DMA-issue
        # queues to halve the startup burst) + transposes ----
        for j in range(G):
            alloc_xr(j)
            for u in range(4):
                emit_load_dma(j, u, eng=(nc.sync if (j + u) % 2 else nc.gpsimd))
        for j in range(G):
            emit_transpose(j)

        pending_stores = []
        for g in range(n_groups):
            s0 = g * G
            H = [x_tiles[(s0 + j) % 8] for j in range(G)]
            # Pump order within this group: next group's loads FIRST (so
            # their transfers complete long before the DVE reaches the
            # matching transposes), then the previous group's stores, then
            # next group's input transposes near mid-group.
            if g + 1 < n_groups:
                for j in range(G):
                    sn = s0 + G + j
                    deferred.append(lambda sn=sn: alloc_xr(sn))
                    for u in range(4):
                        deferred.append(
                            lambda sn=sn, u=u: emit_load_dma(sn, u)
                        )
            deferred.extend(pending_stores)
            pending_stores = []
            if g + 1 < n_groups:
                for j in range(G):
                    sn = s0 + G + j
                    deferred.append(lambda sn=sn: emit_transpose(sn))
            for k in range(N_LAYERS):
                for j in range(G):
                    off = False
                    if k < N_LAYERS - 1:
                        off = counter[0] % offp == 0
                        counter[0] += 1
                    H[j] = layer(H[j], k, off)
                    pump()
            for j in range(G):
                emit_store(s0 + j, H[j], pending_stores)

        deferred.extend(pending_stores)
        while deferred:
            pump()

    nc.compile()
    return nc


def _get_program(n_groups, offp):
    key = (n_groups, offp)
    if key not in _BUILD_CACHE:
        _BUILD_CACHE[key] = _build(n_groups, offp)
    return _BUILD_CACHE[key]


def _pack_weights(W1, Whid, Wout):
    """[128, 24*32]: per partition-group u, column block l*32 holds W_l.T."""
    WT = np.zeros((N_LAYERS, 32, 32), np.float32)
    WT[0, :D_IN, :] = np.asarray(W1, np.float32).T
    WT[1:23] = np.transpose(np.asarray(Whid, np.float32), (0, 2, 1))
    WT[23, :, :3] = np.asarray(Wout, np.float32).T
    Wh = np.zeros((128, N_LAYERS * 32), np.float32)
    blocks = WT.transpose(1, 0, 2).reshape(32, N_LAYERS * 32)
    for u in range(4):
        Wh[32 * u : 32 * u + 32, :] = blocks
    return Wh


def _run(x, W1, Whid, Wout, trace=False, n_groups=None, **spmd_kwargs):
    from concourse.bass_utils import run_bass_kernel_spmd

    if n_groups is None:
        n_groups = int(os.environ.get("BASSK_GROUPS", N_GROUPS))
    offp = int(os.environ.get("BASSK_OFFP", 9))
    nc = _get_program(n_groups, offp)

    x = np.ascontiguousarray(np.asarray(x, np.float32))
    assert x.shape == (N_PIX, D_IN), x.shape
    Wh = _pack_weights(W1, Whid, Wout)

    in_maps = [
        {"x": x[i * P_CORE : (i + 1) * P_CORE], "w": Wh}
        for i in range(N_CORES)
    ]
    res = run_bass_kernel_spmd(
        nc, in_maps, list(range(N_CORES)), trace=trace, **spmd_kwargs
    )
    out = np.concatenate([res.results[i]["o"] for i in range(N_CORES)], axis=0)
    return out, res


def kernel(x, W1, Whid, Wout):
    out, _ = _run(x, W1, Whid, Wout)
    return out


# revision 10
# speedup vs baseline: 1.1983x; 1.1733x over previous
"""Trainium2 Bass kernel for a CPPN-style dense MLP forward pass.

Network (per pixel): 11 -> [32 x 23 tanh layers] -> 3 sigmoid.
  h = tanh(x @ W1.T); 22x: h = tanh(h @ Whid[l].T); out = sigmoid(h @ Wout.T)

Full inputs:  x [4194304, 11] f32, W1 [32, 11], Whid [22, 32, 32], Wout [3, 32]
Full output:  [4194304, 3] f32

Strategy: pure data parallel over 8 NeuronCores (pixels split 8 ways,
weights replicated).  Per core the kernel is ScalarE(tanh)-throughput
bound (24 layers x 32 feats x 512K pixels / 128 lanes at 1.2 GHz =
2.62 ms of pure ACT streaming).  Two levers beyond the ACT floor:

1. ~11% of the tanh tiles are offloaded to the otherwise-idle VectorE
   via a 6-instruction fp32 chain: clamp (tensor_scalar), a (9,4)
   rational x*P(x^2)/D(x^2) evaluated with two custom DVE uop programs
   (8-deep ALU pipe, 1 elem/cycle), the stock bitwise-NOT+2xNR fast
   reciprocal (~51 ULP), and a tensor_tensor multiply.  Max abs error
   1.1e-5 per application; end-to-end L2 ~3e-3 at 1/8 offload (gate
   2e-2).  Offloaded tiles vacate PSUM immediately (the clamp is the
   only PSUM reader), and FOUR supertile streams rotate over the two
   PSUM halves so ACT+PE stay busy during the ~13us DVE windows.

2. Pair-boundary DMA/transpose bursts (which perturbed the PE enough
   to stall ACT ~9% of the time in the 2-stream baseline) are spread:
   loads for the next group, input transposes, output transposes and
   stores are emitted one per (layer,stream) slot via a deferred queue.

Layout per core: pixels in supertiles of 16 tiles x 512 = 8192 pixels.
Activations are feature-major: tile (a,b) holds [32 feats, 512 pixels]
at partitions [32u:32u+32], free offset 512*v, (u,v)=(a,b) on even
layers, (b,a) on odd.  Each layer = 16 concurrent 32x32 matmuls via
tile_position packing writing one [128,2048] PSUM half, then one ACT
tanh (or the DVE chain) to SBUF.  Matmuls are full fp32 (2-pass
LOW/HIGH): the 24-layer tanh chain amplifies per-layer error ~1000x,
so reduced matmul precision (f32r: 0.14 L2) is unusable.

I/O: x loads pixel-major with 44B contiguous chunks, block-transposed
feature-major on VectorE (32x32 STREAM_TRANSPOSE); sigmoid output is
block-transposed back so stores scatter 12B/pixel chunks.  All DMA is
issued from SyncE, spread across layer slots.
"""

import os
import sys

if "/opt/trn_rl_repo" not in sys.path:
    sys.path.insert(0, "/opt/trn_rl_repo")

import numpy as np

N_CORES = 8
N_PIX = 4194304
P_CORE = N_PIX // N_CORES      # 524288 pixels per core
D_IN = 11
D_H = 32
N_LAYERS = 24                  # 1 input + 22 hidden + 1 output
F = 512                        # pixels per tile (one PSUM bank of fp32)
ST_PIX = 16 * F                # 8192 pixels per supertile
N_ST = P_CORE // ST_PIX        # 64 supertiles per core
G = 4                          # supertile streams rotating per group
N_GROUPS = N_ST // G           # 16 groups per core
G_PIX = G * ST_PIX             # 32768 pixels per group

# (9,4) odd rational tanh(x) ~ xc*P(s)/D(s), s=xc^2, xc=clamp(x,+-CLAMP).
# Max abs err 8.1e-6 (fit) / 1.11e-5 (fp32 chain incl ~51ULP reciprocal).
TANH_CLAMP = 7.90531
TA1 = 9.99963490e-01
TA3 = 1.10930869e-01
TA5 = 1.11048222e-03
TA7 = -3.85775571e-06
TA9 = 1.18000171e-08
TB2 = 0.44415242
TB4 = 0.01593033

_BUILD_CACHE = {}
_DVE_OPS = {}


def _register_dve_ops():
    """Register the two custom DVE uop programs used by the tanh chain.

    POLY2_ANT: out = (s*C0 + C1)*s + C2, s = in0^2   (numerator partial
      with (C0,C1,C2)=(a9,a7,a5); denominator with (b4,b2,1.0))
    TANH_NUMB_ANT: out = ((in1*s + C0)*s + C1)*in0, s = in0^2
      (finishes the odd numerator: N = ((A*s+a3)*s+a1)*xc)
    """
    if _DVE_OPS:
        return _DVE_OPS
    import concourse.dve_ops as dve_ops
    from concourse.dve_spec import Spec, Src0, Src1, C0, C1, C2, sq, lower
    from concourse.dve_spec import _has_src1
    from concourse.dve_uop import DveOpSpec

    def make(name, spec):
        shas = {}
        for ver in ("v3", "v4"):
            d = DveOpSpec(
                name=name, opcode=1, uops=lower(spec, ver=ver),
                rd1_en=_has_src1(spec),
            )
            shas[ver] = d.sha(ver)
        op = dve_ops.DveOp(name=name, spec=spec, subdim=False, uops_sha=shas)
        if name not in dve_ops._SUB_OPCODE_FOR_NAME:
            dve_ops.OPS.append(op)
            dve_ops._SUB_OPCODE_FOR_NAME[name] = (
                dve_ops._CUSTOM_DVE_ROW_BASE + len(dve_ops.OPS) - 1
            )
            dve_ops.CUSTOM_DVE_SPECS[name] = spec
        assert dve_ops._SUB_OPCODE_FOR_NAME[name] < 0x20
        return op

    s = sq(Src0)
    poly2 = Spec(
        body=(s * C0 + C1) * s + C2,
        reference=lambda in0, in1, c0, c1, c2: (
            ((in0 * in0) * c0 + c1) * (in0 * in0) + c2
        ),
    )
    s2 = sq(Src0)
    numb = Spec(
        body=((Src1 * s2 + C0) * s2 + C1) * Src0,
        reference=lambda in0, in1, c0, c1, c2: (
            ((in1 * (in0 * in0) + c0) * (in0 * in0) + c1) * in0
        ),
    )
    _DVE_OPS["poly2"] = make("POLY2_ANT", poly2)
    _DVE_OPS["numb"] = make("TANH_NUMB_ANT", numb)
    return _DVE_OPS


def _build(n_groups, offp):
    """Build + bass-compile the per-core program. Returns the Bacc object."""
    import concourse.bass as bass  # noqa: F401
    import concourse.tile as tile
    from concourse import bacc, mybir
    from concourse.alu_op_type import AluOpType
    from contextlib import ExitStack

    ops = _register_dve_ops()
    f32 = mybir.dt.float32
    Tanh = mybir.ActivationFunctionType.Tanh
    Sigmoid = mybir.ActivationFunctionType.Sigmoid

    nc = bacc.Bacc(
        "TRN2", target_bir_lowering=False, debug=False, num_devices=N_CORES
    )
    x_ap = nc.dram_tensor("x", [P_CORE, D_IN], f32, kind="ExternalInput").ap()
    w_ap = nc.dram_tensor("w", [128, N_LAYERS * 32], f32, kind="ExternalInput").ap()
    o_ap = nc.dram_tensor("o", [P_CORE, 3], f32, kind="ExternalOutput").ap()

    with tile.TileContext(nc) as tc, ExitStack() as ctx:
        wp = ctx.enter_context(tc.tile_pool(name="wp", bufs=1))
        xrp = ctx.enter_context(tc.tile_pool(name="xrp", bufs=4))
        xp = ctx.enter_context(tc.tile_pool(name="xp", bufs=4))
        hp = ctx.enter_context(tc.tile_pool(name="hp", bufs=5))
        op_ = ctx.enter_context(tc.tile_pool(name="op", bufs=4))
        srp = ctx.enter_context(tc.tile_pool(name="srp", bufs=2))
        cp = ctx.enter_context(tc.tile_pool(name="cp", bufs=5))
        pp = ctx.enter_context(tc.tile_pool(name="pp", bufs=2, space="PSUM"))

        Wf = wp.tile([128, N_LAYERS * 32], f32)
        nc.sync.dma_start(Wf[:], w_ap[:])

        # ---- deferred work queue: one item per (layer, stream) slot ----
        deferred = []

        def pump():
            if deferred:
                deferred.pop(0)()

        def emit_load_dma(s, u, eng=None):
            # Pixel-major load, 44B contiguous chunks per pixel row, laid
            # out so a 32x32 block-transpose yields feature-major tiles:
            # XR[32u+p, 32c+f] = x[s*8192 + u*2048 + 32c + p, f].
            XR = xr_tiles[s % 8]
            p0 = s * ST_PIX + u * 2048
            (eng or nc.sync).dma_start(
                XR[32 * u : 32 * u + 32, :].rearrange(
                    "p (c f) -> p c f", c=64, f=32
                )[:, :, 0:D_IN],
                x_ap[p0 : p0 + 2048, :].rearrange("(c p) f -> p c f", c=64, p=32),
            )

        def emit_transpose(s):
            # DVE 32x32 block transpose -> X[32u+f, 32c+p].
            X = xp.tile([128, 2048], f32, name="X", tag="X")
            nc.vector.transpose(X[:], xr_tiles[s % 8][:])
            x_tiles[s % 8] = X

        xr_tiles = {}
        x_tiles = {}

        def alloc_xr(s):
            xr_tiles[s % 8] = xrp.tile([128, 2048], f32, name="XR", tag="XR")

        def emit_store(s, S, out):
            # Block-transpose back to pixel-major; 12B/pixel store chunks.
            # Both the transpose and the 4 store DMAs are deferred so they
            # spread across the next group's layer slots.  Stores issue on
            # the (otherwise idle) GpSimd queue so they never delay the
            # next group's input loads on SyncE.
            box = {}

            def tp(S=S, box=box):
                SR = srp.tile([128, 2048], f32, name="SR", tag="SR")
                nc.vector.transpose(SR[:], S[:])
                box["SR"] = SR

            out.append(tp)
            for a in range(4):
                p0 = s * ST_PIX + a * 2048

                def st(a=a, p0=p0, box=box):
                    SR = box["SR"]
                    nc.gpsimd.dma_start(
                        o_ap[p0 : p0 + 2048, :].rearrange(
                            "(c p) f -> p c f", c=64, p=32
                        ),
                        SR[32 * a : 32 * a + 32, :].rearrange(
                            "p (c f) -> p c f", c=64, f=32
                        )[:, :, 0:3],
                    )

                out.append(st)

        counter = [0]

        def layer(H, k, offload):
            """One layer for one supertile: 16 packed matmuls + tanh."""
            Kd = D_IN if k == 0 else 32
            last = k == N_LAYERS - 1
            P_ = pp.tile([128, 2048], f32)
            # fp32 16-way tile-position packing; iterate so consecutive
            # matmuls land on different PE row groups (LDWEIGHTS only
            # pulls ahead of in-flight MMs when row_grp differs).
            ab = [(a, b) for b in range(4) for a in range(4)]
            if k % 2 == 1:
                ab = [(a, b) for a in range(4) for b in range(4)]
            for a, b in ab:
                u, v = (a, b) if k % 2 == 0 else (b, a)
                nc.tensor.matmul(
                    P_[32 * v : 32 * v + 32, 512 * u : 512 * u + 512],
                    lhsT=Wf[32 * u : 32 * u + Kd, 32 * k : 32 * k + 32],
                    rhs=H[32 * u : 32 * u + Kd, 512 * v : 512 * v + 512],
                    start=True,
                    stop=True,
                    tile_position=(32 * u, 32 * v),
                )
            if last:
                Hn = op_.tile([128, 2048], f32)
                nc.scalar.activation(Hn[:], P_[:], Sigmoid)
            elif offload:
                mn, mx = AluOpType.min, AluOpType.max
                xc = cp.tile([128, 2048], f32, name="xc", tag="chain")
                nc.vector.tensor_scalar(
                    xc[:], P_[:], float(TANH_CLAMP), float(-TANH_CLAMP), mn, mx
                )
                A = cp.tile([128, 2048], f32, name="A", tag="chain")
                nc.vector._custom_dve(
                    ops["poly2"], out=A[:], in0=xc[:],
                    s0=float(TA9), s1=float(TA7), imm2=float(TA5),
                )
                Nt = cp.tile([128, 2048], f32, name="Nt", tag="chain")
                nc.vector._custom_dve(
                    ops["numb"], out=Nt[:], in0=xc[:], in1=A[:],
                    s0=float(TA3), s1=float(TA1),
                )
                Dt = cp.tile([128, 2048], f32, name="Dt", tag="chain")
                nc.vector._custom_dve(
                    ops["poly2"], out=Dt[:], in0=xc[:],
                    s0=float(TB4), s1=float(TB2), imm2=1.0,
                )
                Rt = cp.tile([128, 2048], f32, name="Rt", tag="chain")
                nc.vector.reciprocal_approx_fast(Rt[:], Dt[:])
                Hn = hp.tile([128, 2048], f32)
                nc.vector.tensor_tensor(Hn[:], Nt[:], Rt[:], AluOpType.mult)
            else:
                Hn = hp.tile([128, 2048], f32)
                nc.scalar.activation(Hn[:], P_[:], Tanh)
            return Hn

        # ---- prologue: group 0 loads (split across both DMA-issue
        # queues to halve the startup burst) + transposes ----
        for j in range(G):
            alloc_xr(j)
            for u in range(4):
                emit_load_dma(j, u, eng=(nc.sync if (j + u) % 2 else nc.gpsimd))
        for j in range(G):
            emit_transpose(j)

        # ---- virtual-time lane scheduler ----
        # Four independent lanes, lane L processing supertiles L, L+4, ...
        # Every engine queue executes in EMISSION order, so a matmul that
        # waits on a slow DVE-chain output would head-of-line-block every
        # other lane's ready matmuls if emitted at its natural round-robin
        # slot.  Instead each lane carries a virtual clock (units of one
        # ACT tile); an offloaded tile advances it by the ~7x longer chain
        # latency, and we always emit next the lane with the smallest
        # clock — so dependent work lands in the queues roughly when its
        # inputs are ready and the queues stay ready-in-order.
        ACT_T, CHAIN_T = 1.0, 7.0
        sup = list(range(G))           # current supertile per lane
        kptr = [0] * G
        clocks = [0.0] * G
        H = [x_tiles[j % 8] for j in range(G)]
        n_sup = n_groups * G

        while True:
            live = [L for L in range(G) if sup[L] < n_sup]
            if not live:
                break
            L = min(live, key=lambda L: clocks[L])
            k = kptr[L]
            s = sup[L]
            if k == 0 and s + G < n_sup:
                # queue next supertile's loads now, its transpose mid-way
                deferred.append(lambda sn=s + G: alloc_xr(sn))
                for u in range(4):
                    deferred.append(lambda sn=s + G, u=u: emit_load_dma(sn, u))
            off = False
            if k < N_LAYERS - 1:
                off = counter[0] % offp == 0
                counter[0] += 1
            H[L] = layer(H[L], k, off)
            pump()
            clocks[L] += CHAIN_T if off else ACT_T
            kptr[L] += 1
            if kptr[L] == 10 and s + G < n_sup:
                deferred.append(lambda sn=s + G: emit_transpose(sn))
            if kptr[L] == N_LAYERS:
                emit_store(s, H[L], deferred)
                sup[L] += G
                kptr[L] = 0
                if sup[L] < n_sup:
                    H[L] = x_tiles[sup[L] % 8]

        while deferred:
            pump()

    nc.compile()
    return nc


def _get_program(n_groups, offp):
    key = (n_groups, offp)
    if key not in _BUILD_CACHE:
        _BUILD_CACHE[key] = _build(n_groups, offp)
    return _BUILD_CACHE[key]


def _pack_weights(W1, Whid, Wout):
    """[128, 24*32]: per partition-group u, column block l*32 holds W_l.T."""
    WT = np.zeros((N_LAYERS, 32, 32), np.float32)
    WT[0, :D_IN, :] = np.asarray(W1, np.float32).T
    WT[1:23] = np.transpose(np.asarray(Whid, np.float32), (0, 2, 1))
    WT[23, :, :3] = np.asarray(Wout, np.float32).T
    Wh = np.zeros((128, N_LAYERS * 32), np.float32)
    blocks = WT.transpose(1, 0, 2).reshape(32, N_LAYERS * 32)
    for u in range(4):
        Wh[32 * u : 32 * u + 32, :] = blocks
    return Wh


def _run(x, W1, Whid, Wout, trace=False, n_groups=None, **spmd_kwargs):
    from concourse.bass_utils import run_bass_kernel_spmd

    if n_groups is None:
        n_groups = int(os.environ.get("BASSK_GROUPS", N_GROUPS))
    offp = int(os.environ.get("BASSK_OFFP", 9))
    nc = _get_program(n_groups, offp)

    x = np.ascontiguousarray(np.asarray(x, np.float32))
    assert x.shape == (N_PIX, D_IN), x.shape
    Wh = _pack_weights(W1, Whid, Wout)

    in_maps = [
        {"x": x[i * P_CORE : (i + 1) * P_CORE], "w": Wh}
        for i in range(N_CORES)
    ]
    res = run_bass_kernel_spmd(
        nc, in_maps, list(range(N_CORES)), trace=trace, **spmd_kwargs
    )
    out = np.concatenate([res.results[i]["o"] for i in range(N_CORES)], axis=0)
    return out, res


def kernel(x, W1, Whid, Wout):
    out, _ = _run(x, W1, Whid, Wout)
    return out


# revision 13
# speedup vs baseline: 1.5141x; 1.2636x over previous
"""Trainium2 Bass kernel for a CPPN-style dense MLP forward pass.

Network (per pixel): 11 -> [32 x 23 tanh layers] -> 3 sigmoid.
  h = tanh(x @ W1.T); 22x: h = tanh(h @ Whid[l].T); out = sigmoid(h @ Wout.T)

Full inputs:  x [4194304, 11] f32, W1 [32, 11], Whid [22, 32, 32], Wout [3, 32]
Full output:  [4194304, 3] f32

Strategy: pure data parallel over 8 NeuronCores (pixels split 8 ways,
weights replicated).  Per core the kernel is ScalarE(tanh)-throughput
bound, so the layout keeps ACT ~100% busy on large [128, 2048]
activations while the PE runs the 32x32 matmuls 16-at-a-time via
tile_position packing (all 16 32x32 sub-arrays concurrently).

Layout per core: pixels processed in "supertiles" of 16 tiles x 512
pixels = 8192 pixels.  Activations live feature-major: tile (a,b) holds
[32 features, 512 pixels] at SBUF partitions [32u:32u+32], free offset
512*v, where (u,v)=(a,b) on even layers and (b,a) on odd layers.  Each
layer = 16 concurrent matmuls at tile_position (32u, 32v) writing one
[128, 2048] PSUM half (4 banks), then one big ACT tanh PSUM->SBUF.
Two supertile streams are interleaved (PSUM ping-pong) so the PE fills
one PSUM half while ACT drains the other.

I/O avoids small-packet DMA death: x is loaded pixel-major with 44B
contiguous chunks and block-transposed to feature-major on the (idle)
VectorE via its 32x32 STREAM_TRANSPOSE; the sigmoid output is
block-transposed back so the store scatters 12B/pixel chunks with a
32-row outer dim (spreads across all 16 DMA engines).  All DMAs are
issued from SyncE - DMA issue occupies the issuing engine's
instruction stream and must stay off the ACT critical path.

Matmuls are full fp32 (2-pass LOW/HIGH).  The 24-layer tanh chain is
chaotic (Lyapunov growth ~700x): fp32 implementations already differ
from each other by ~2e-4 L2 on the final output, and reduced matmul
precision (float32r, ~12 mantissa bits, would be 2x faster and
single-pass) amplifies to ~0.14 L2 - unusable.  Measured: ~3.20 ms on
hardware, vs a 2.95 ms ScalarE floor (1536 ACTIVATEs x (2048+222)cyc
@ 1.2 GHz); PE/DVE/DMA are all hidden under the tanh stream.
"""

import os
import sys

if "/opt/trn_rl_repo" not in sys.path:
    sys.path.insert(0, "/opt/trn_rl_repo")

import numpy as np

N_CORES = 8
N_PIX = 4194304
P_CORE = N_PIX // N_CORES      # 524288 pixels per core
D_IN = 11
D_H = 32
N_LAYERS = 24                  # 1 input + 22 hidden + 1 output
F = 512                        # pixels per tile (one PSUM bank of fp32)
ST_PIX = 16 * F                # 8192 pixels per supertile
N_ST = P_CORE // ST_PIX        # 64 supertiles per core
N_PAIRS = N_ST // 2            # 32 interleaved supertile pairs

_BUILD_CACHE = {}


def _build(n_pairs):
    """Build + bass-compile the per-core program. Returns the Bacc object."""
    import concourse.bass as bass  # noqa: F401
    import concourse.tile as tile
    from concourse import bacc, mybir
    from contextlib import ExitStack

    f32 = mybir.dt.float32
    Tanh = mybir.ActivationFunctionType.Tanh
    Sigmoid = mybir.ActivationFunctionType.Sigmoid

    nc = bacc.Bacc(
        "TRN2", target_bir_lowering=False, debug=False, num_devices=N_CORES
    )
    x_ap = nc.dram_tensor("x", [P_CORE, D_IN], f32, kind="ExternalInput").ap()
    w_ap = nc.dram_tensor("w", [128, N_LAYERS * 32], f32, kind="ExternalInput").ap()
    wbd_ap = nc.dram_tensor("wbd", [128, 22 * 128], f32, kind="ExternalInput").ap()
    o_ap = nc.dram_tensor("o", [P_CORE, 3], f32, kind="ExternalOutput").ap()

    with tile.TileContext(nc) as tc, ExitStack() as ctx:
        wp = ctx.enter_context(tc.tile_pool(name="wp", bufs=1))
        xrp = ctx.enter_context(tc.tile_pool(name="xrp", bufs=4))
        xp = ctx.enter_context(tc.tile_pool(name="xp", bufs=4))
        hp = ctx.enter_context(tc.tile_pool(name="hp", bufs=4))
        sp = ctx.enter_context(tc.tile_pool(name="sp", bufs=6))
        pp = ctx.enter_context(tc.tile_pool(name="pp", bufs=2, space="PSUM"))

        mm_dt = (
            mybir.dt.float32r
            if os.environ.get("BASSK_MMDT", "f32") == "f32r"
            else f32
        )

        Wf = wp.tile([128, N_LAYERS * 32], f32)
        nc.sync.dma_start(Wf[:], w_ap[:])
        if mm_dt != f32:
            # Hidden layers run as full-array [128,128] block-diagonal f32r
            # matmuls (f32r only supports column-group 0, so no 16-way
            # packing); the explicit scalar copy is the required f32r
            # rounding producer.
            Wbf = wp.tile([128, 22 * 128], f32)
            nc.sync.dma_start(Wbf[:], wbd_ap[:])
            Wbr = wp.tile([128, 22 * 128], mm_dt)
            nc.scalar.copy(Wbr[:], Wbf[:])

        # ---- deferred work queue: one item per (layer, stream) slot.
        # The x loads for pair p+1, the stores of pair p-1 and the input
        # transposes are spread across pair p's 48 layer slots instead of
        # bursting at the pair boundary - the clustered DMA traffic +
        # transposes perturbed the PE enough there to stall ACT ~9% of
        # the time.  Order per pair: loads FIRST (so transfers are long
        # done before the DVE reaches the matching transposes), then the
        # previous pair's stores (issued on the otherwise-idle GpSimd
        # queue so they never delay loads on SyncE), then the transposes.
        deferred = []
        xr_tiles = {}
        x_tiles = {}

        def pump():
            if deferred:
                deferred.pop(0)()

        def alloc_xr(s):
            xr_tiles[s % 4] = xrp.tile([128, 2048], f32, name="XR", tag="XR")

        def emit_load_dma(s, u, eng=None):
            # Pixel-major load, 44B contiguous chunks per pixel row, laid
            # out so that a 32x32 block-transpose yields feature-major
            # tiles: XR[32u+p, 32c+f] = x[s*8192 + u*2048 + 32c + p, f].
            XR = xr_tiles[s % 4]
            p0 = s * ST_PIX + u * 2048
            (eng or nc.sync).dma_start(
                XR[32 * u : 32 * u + 32, :].rearrange(
                    "p (c f) -> p c f", c=64, f=32
                )[:, :, 0:D_IN],
                x_ap[p0 : p0 + 2048, :].rearrange("(c p) f -> p c f", c=64, p=32),
            )

        def emit_transpose(s):
            # DVE 32x32 block transpose -> X[32u+f, 32c+p].
            X = xp.tile([128, 2048], f32, name="X", tag="X")
            nc.vector.transpose(X[:], xr_tiles[s % 4][:])
            x_tiles[s % 4] = X


        def layer(H, k):
            """One layer for one supertile: 16 packed matmuls + one ACT."""
            Kd = D_IN if k == 0 else 32
            last = k == N_LAYERS - 1
            P_ = pp.tile([128, 2048], f32)
            if mm_dt != f32 and 1 <= k <= 22:
                # Hidden layer: 4 block-diagonal full-array f32r matmuls,
                # layout-preserving: tile (a,g) stays at [32g, 512a].
                for a in range(4):
                    nc.tensor.matmul(
                        P_[:, 512 * a : 512 * a + 512],
                        lhsT=Wbr[:, 128 * (k - 1) : 128 * k],
                        rhs=H[:, 512 * a : 512 * a + 512],
                        start=True,
                        stop=True,
                        tile_position=(0, 0),
                    )
            else:
                # fp32 16-way tile-position packing; iterate so consecutive
                # matmuls land on different PE row groups (LDWEIGHTS only
                # pulls ahead of in-flight MMs when row_grp differs).
                ab = [(a, b) for b in range(4) for a in range(4)]
                if k % 2 == 1:
                    ab = [(a, b) for a in range(4) for b in range(4)]
                for a, b in ab:
                        u, v = (a, b) if k % 2 == 0 else (b, a)
                        nc.tensor.matmul(
                            P_[32 * v : 32 * v + 32, 512 * u : 512 * u + 512],
                            lhsT=Wf[32 * u : 32 * u + Kd, 32 * k : 32 * k + 32],
                            rhs=H[32 * u : 32 * u + Kd, 512 * v : 512 * v + 512],
                            start=True,
                            stop=True,
                            tile_position=(32 * u, 32 * v),
                        )
            if last:
                Hn = sp.tile([128, 2048], f32)
                nc.scalar.activation(Hn[:], P_[:], Sigmoid)
            else:
                # Layer 22's output feeds the fp32 16-way output layer.
                h_dt = f32 if k == N_LAYERS - 2 else mm_dt
                Hn = hp.tile([128, 2048], h_dt)
                nc.scalar.activation(Hn[:], P_[:], Tanh)
            return Hn

        def emit_store(s, S, out):
            # Block-transpose back to pixel-major so the scatter uses 12B
            # chunks with a 32-row outer dim (spreads across all DMA engines):
            # SR[32a+p, 32c+f] = S[32a+f, 32c+p] = out feature f of pixel
            # s*8192 + a*2048 + 32c + p.  Transpose + 4 store DMAs are all
            # deferred into the next pair's layer slots.
            box = {}

            def tp(S=S, box=box):
                SR = sp.tile([128, 2048], f32, name="SR", tag="SR")
                nc.vector.transpose(SR[:], S[:])
                box["SR"] = SR

            out.append(tp)
            for a in range(4):
                p0 = s * ST_PIX + a * 2048

                def st(a=a, p0=p0, box=box):
                    SR = box["SR"]
                    nc.gpsimd.dma_start(
                        o_ap[p0 : p0 + 2048, :].rearrange(
                            "(c p) f -> p c f", c=64, p=32
                        ),
                        SR[32 * a : 32 * a + 32, :].rearrange(
                            "p (c f) -> p c f", c=64, f=32
                        )[:, :, 0:3],
                    )

                out.append(st)

        # prologue: pair 0 loads split across both DMA-issue queues
        for s in (0, 1):
            alloc_xr(s)
            for u in range(4):
                emit_load_dma(s, u, eng=(nc.sync if (s + u) % 2 else nc.gpsimd))
        for s in (0, 1):
            emit_transpose(s)

        pending_stores = []
        for pair in range(n_pairs):
            sA, sB = 2 * pair, 2 * pair + 1
            HA, HB = x_tiles[sA % 4], x_tiles[sB % 4]
            if pair + 1 < n_pairs:
                for sn in (sA + 2, sB + 2):
                    deferred.append(lambda sn=sn: alloc_xr(sn))
                    for u in range(4):
                        deferred.append(lambda sn=sn, u=u: emit_load_dma(sn, u))
            deferred.extend(pending_stores)
            pending_stores = []
            if pair + 1 < n_pairs:
                for sn in (sA + 2, sB + 2):
                    deferred.append(lambda sn=sn: emit_transpose(sn))
            # Interleave the two streams layer-by-layer so the PSUM pool's
            # two slots ping-pong A/B and ACT never waits on the PE.
            for k in range(N_LAYERS):
                HA = layer(HA, k)
                pump()
                HB = layer(HB, k)
                pump()
            emit_store(sA, HA, pending_stores)
            emit_store(sB, HB, pending_stores)

        deferred.extend(pending_stores)
        while deferred:
            pump()

    nc.compile()
    return nc


def _get_program(n_pairs):
    if n_pairs not in _BUILD_CACHE:
        _BUILD_CACHE[n_pairs] = _build(n_pairs)
    return _BUILD_CACHE[n_pairs]


def _pack_weights(W1, Whid, Wout):
    """[128, 24*32]: per partition-group u, column block l*32 holds W_l.T."""
    WT = np.zeros((N_LAYERS, 32, 32), np.float32)
    WT[0, :D_IN, :] = np.asarray(W1, np.float32).T
    WT[1:23] = np.transpose(np.asarray(Whid, np.float32), (0, 2, 1))
    WT[23, :, :3] = np.asarray(Wout, np.float32).T
    Wh = np.zeros((128, N_LAYERS * 32), np.float32)
    blocks = WT.transpose(1, 0, 2).reshape(32, N_LAYERS * 32)
    for u in range(4):
        Wh[32 * u : 32 * u + 32, :] = blocks
    Wbd = np.zeros((128, 22, 128), np.float32)
    for g in range(4):
        Wbd[32 * g : 32 * g + 32, :, 32 * g : 32 * g + 32] = WT[1:23].transpose(
            1, 0, 2
        )
    return Wh, Wbd.reshape(128, 22 * 128)


def _run(x, W1, Whid, Wout, trace=False, n_pairs=None, **spmd_kwargs):
    from concourse.bass_utils import run_bass_kernel_spmd

    if n_pairs is None:
        n_pairs = int(os.environ.get("BASSK_PAIRS", N_PAIRS))
    nc = _get_program(n_pairs)

    x = np.ascontiguousarray(np.asarray(x, np.float32))
    assert x.shape == (N_PIX, D_IN), x.shape
    Wh, Wbd = _pack_weights(W1, Whid, Wout)

    in_maps = [
        {"x": x[i * P_CORE : (i + 1) * P_CORE], "w": Wh, "wbd": Wbd}
        for i in range(N_CORES)
    ]
    res = run_bass_kernel_spmd(
        nc, in_maps, list(range(N_CORES)), trace=trace, **spmd_kwargs
    )
    out = np.concatenate([res.results[i]["o"] for i in range(N_CORES)], axis=0)
    return out, res


def kernel(x, W1, Whid, Wout):
    out, _ = _run(x, W1, Whid, Wout)
    return out
